# revision 1
# baseline (speedup 1.0000x reference)
"""Content-addressed cache-select kernel for Trainium2 (8 NeuronCores, SPMD).

Problem: out = cached_outputs[idx] where idx is the first row of
`fingerprints` (6x4) exactly equal to the first 4 floats of `x`, else 0.

Strategy (row-parallel over 8 cores):
  - Each core receives its 2048-row shard of all 6 cached slabs, plus the
    (replicated) 4-float probe and the 6x4 fingerprint table.
  - On device, the SP engine stages probe+fingerprints into SBUF, computes
    the match index with register ops (bitwise f32 equality, first match
    wins, no match -> 0), then issues dynamic-offset DRAM->DRAM DMAs that
    copy the selected 32MB slab shard to the output.
"""
import numpy as np

import concourse.bass as bass
import concourse.mybir as mybir
from concourse.bass_utils import run_bass_kernel_spmd

N_CASES = 6
ROWS, COLS = 16384, 4096
N_CORES = 8
RS = ROWS // N_CORES  # rows per core
CHUNK_ROWS = 512  # 512*4096*4B = 8MB per DMA


def build(chunk_rows=CHUNK_ROWS):
    nc = bass.Bass()
    f32 = mybir.dt.float32
    i32 = mybir.dt.int32

    probe = nc.dram_tensor("probe", [1, 4], f32, kind="ExternalInput")
    fps = nc.dram_tensor("fps", [N_CASES, 4], f32, kind="ExternalInput")
    cached = nc.dram_tensor("cached", [N_CASES, RS, COLS], f32, kind="ExternalInput")
    out = nc.dram_tensor("out", [RS, COLS], f32, kind="ExternalOutput")

    n_chunks = RS // chunk_rows
    assert RS % chunk_rows == 0

    with (
        nc.sbuf_tensor("stage", [1, 32], f32) as stage,
        nc.Block() as block,
        nc.semaphore("ssem") as ssem,
        nc.semaphore("bsem") as bsem,
    ):

        @block.sync
        def _(sync):
            # Stage probe (4 floats) + fingerprints (24 floats) into SBUF.
            sync.dma_start(stage[0:1, 0:4], probe[0:1, 0:4]).then_inc(ssem, 16)
            sync.dma_start(stage[0:1, 4:28], fps[:, :]).then_inc(ssem, 16)
            sync.wait_ge(ssem, 32)

            stage_i = stage[:].bitcast(i32)

            with (
                sync.register("p0") as p0,
                sync.register("p1") as p1,
                sync.register("p2") as p2,
                sync.register("p3") as p3,
                sync.register("t") as t,
                sync.register("m") as m,
                sync.register("idx") as idx_reg,
            ):
                pregs = [p0, p1, p2, p3]
                for j in range(4):
                    sync.reg_load(pregs[j], stage_i[0:1, j : j + 1])
                sync.reg_mov(idx_reg, 0)
                # Scan cases last-to-first so the lowest matching index wins.
                for i in reversed(range(N_CASES)):
                    for j in range(4):
                        sync.reg_load(t, stage_i[0:1, 4 + 4 * i + j : 5 + 4 * i + j])
                        sync.reg_alu(t, t, pregs[j], mybir.AluOpType.is_equal)
                        if j == 0:
                            sync.reg_mov(m, t)
                        else:
                            sync.reg_alu(m, m, t, mybir.AluOpType.bitwise_and)
                    # idx = m ? i : idx  ==  idx + m*(i - idx)
                    sync.reg_alu(t, i, idx_reg, mybir.AluOpType.subtract)
                    sync.reg_alu(t, t, m, mybir.AluOpType.mult)
                    sync.reg_alu(idx_reg, idx_reg, t, mybir.AluOpType.add)

                idx = sync.snap(idx_reg, min_val=0, max_val=N_CASES - 1)

            # Data-dependent DRAM->DRAM copy of the selected slab shard.
            for k in range(n_chunks):
                rows = slice(k * chunk_rows, (k + 1) * chunk_rows)
                sync.dma_start(out[rows, :], cached[bass.ds(idx, 1), rows, :]).then_inc(
                    bsem, 16
                )
            sync.wait_ge(bsem, 16 * n_chunks)

    return nc


def run(inputs, trace=False, **spmd_kwargs):
    x = np.asarray(inputs["x"], dtype=np.float32)
    fingerprints = np.asarray(inputs["fingerprints"], dtype=np.float32)
    cached_outputs = np.asarray(inputs["cached_outputs"], dtype=np.float32)

    nc = build()
    probe = np.ascontiguousarray(x.reshape(-1)[:4].reshape(1, 4))
    fps = np.ascontiguousarray(fingerprints)
    in_maps = []
    for c in range(N_CORES):
        shard = np.ascontiguousarray(cached_outputs[:, c * RS : (c + 1) * RS, :])
        in_maps.append({"probe": probe, "fps": fps, "cached": shard})

    res = run_bass_kernel_spmd(nc, in_maps, list(range(N_CORES)), trace=trace, **spmd_kwargs)
    out = np.concatenate([res.results[c]["out"] for c in range(N_CORES)], axis=0)
    return out.astype(np.float32), res


def kernel(**inputs) -> np.ndarray:
    out, _ = run(inputs, trace=False)
    return out


# revision 3
# speedup vs baseline: 1.2862x; 1.2862x over previous
"""Content-addressed cache-select kernel for Trainium2 (8 NeuronCores, SPMD).

Problem: out = cached_outputs[idx] where idx is the first row of
`fingerprints` (6x4) exactly equal to the first 4 floats of `x`, else 0.

Strategy (row-parallel over 8 cores):
  - Each core receives its 2048-row shard of all 6 cached slabs plus a
    small staged "meta" vector (fingerprints, the replicated probe tiled
    x6, and index weights) packed on the host.
  - On device: one DMA stages meta into SBUF; the vector engine computes
    the first-match index in 6 small int32 ops (bitwise equality ==
    float equality for these inputs; first match wins, no match -> 0);
    the SP engine loads the index into a register and issues
    dynamic-offset DRAM->DRAM DMAs copying the selected 32MB slab shard
    to the output.
  - The copy is SDMA-engine-bound (~3.16us per 64KB descriptor pair per
    engine). Descriptors round-robin across the 16 SDMA engines per DMA
    instruction; the chunk schedule below skews work away from SDMA
    engine 15, which measures ~12% slower (known TRN2 trait).
"""
import numpy as np

import concourse.bass as bass
import concourse.mybir as mybir
from concourse.bass_utils import run_bass_kernel_spmd

N_CASES = 6
ROWS, COLS = 16384, 4096
N_CORES = 8
RS = ROWS // N_CORES  # rows per core

# Copy chunk schedule in rows (4096 f32 = 16KB per row; 4 rows = one 64KB
# descriptor). 508 rows = 127 descriptors -> engines 0-14 get 8, engine 15
# gets 7 per DMA instruction.
CHUNKS_UNIFORM = [512, 512, 512, 512]
CHUNKS_SKEW = [508, 508, 508, 508, 16]
CHUNKS = CHUNKS_SKEW


def build(chunks=None):
    chunks = chunks or CHUNKS
    assert sum(chunks) == RS
    nc = bass.Bass()
    f32 = mybir.dt.float32
    i32 = mybir.dt.int32

    meta = nc.dram_tensor("meta", [1, 64], i32, kind="ExternalInput")
    cached = nc.dram_tensor("cached", [N_CASES, RS, COLS], f32, kind="ExternalInput")
    out = nc.dram_tensor("out", [RS, COLS], f32, kind="ExternalOutput")

    with (
        nc.sbuf_tensor("stage", [1, 128], i32) as stage,
        nc.Block(no_gpsimd_drain=True) as block,
        nc.semaphore("ssem") as ssem,
        nc.semaphore("vsem") as vsem,
        nc.semaphore("bsem") as bsem,
    ):

        @block.sync
        def _(sync):
            sync.dma_start(stage[0:1, 0:64], meta[0:1, 0:64]).then_inc(ssem, 16)

        @block.vector
        def _(vector):
            vector.wait_ge(ssem, 16)
            st = stage
            step = [0]

            def chain(inst):
                step[0] += 1
                inst.then_inc(vsem, 1)
                vector.wait_ge(vsem, step[0])

            # eq[64:88] = (fps == probe_tiled) as int32 0/1 (bitwise equality)
            chain(
                vector.tensor_tensor(
                    st[0:1, 64:88],
                    st[0:1, 0:24],
                    st[0:1, 24:48],
                    mybir.AluOpType.is_equal,
                )
            )
            # all4[88:94] = min over each fingerprint's 4 equality bits
            eq_v = st[0:1, 64:88].rearrange("p (a b) -> p a b", a=6)
            chain(
                vector.tensor_reduce(
                    st[0:1, 88:94], eq_v, mybir.AxisListType.X, mybir.AluOpType.min
                )
            )
            # score[94:100] = all4 * [8,7,6,5,4,3] (weights staged at [48:54])
            chain(
                vector.tensor_tensor(
                    st[0:1, 94:100],
                    st[0:1, 88:94],
                    st[0:1, 48:54],
                    mybir.AluOpType.mult,
                )
            )
            # m[100:101] = max(score) = 8 - first_match (0 if no match)
            chain(
                vector.tensor_reduce(
                    st[0:1, 100:101],
                    st[0:1, 94:100],
                    mybir.AxisListType.X,
                    mybir.AluOpType.max,
                )
            )
            # t[101:102] = 8 - m ; idx[102:103] = t & 7 (maps no-match 8 -> 0)
            chain(
                vector.tensor_scalar(
                    st[0:1, 101:102],
                    st[0:1, 100:101],
                    -1,
                    8,
                    mybir.AluOpType.mult,
                    mybir.AluOpType.add,
                )
            )
            chain(
                vector.tensor_scalar(
                    st[0:1, 102:103],
                    st[0:1, 101:102],
                    7,
                    None,
                    mybir.AluOpType.bitwise_and,
                )
            )

        @block.sync
        def _(sync):
            sync.wait_ge(vsem, 6)
            with sync.register("idxr") as idxr:
                sync.reg_load(idxr, stage[0:1, 102:103])
                idx = sync.snap(idxr, min_val=0, max_val=N_CASES - 1)
            n = 0
            r0 = 0
            for rows_n in chunks:
                rows = slice(r0, r0 + rows_n)
                sync.dma_start(out[rows, :], cached[bass.ds(idx, 1), rows, :]).then_inc(
                    bsem, 16
                )
                r0 += rows_n
                n += 1
            sync.wait_ge(bsem, 16 * n)

    return nc


def make_meta(probe, fps):
    buf = np.zeros((1, 64), dtype=np.int32)
    buf[0, 0:24] = fps.reshape(-1).view(np.int32)
    buf[0, 24:48] = np.tile(probe.reshape(-1), 6).view(np.int32)
    buf[0, 48:54] = np.array([8, 7, 6, 5, 4, 3], dtype=np.int32)
    return buf


def run(inputs, trace=False, chunks=None, **spmd_kwargs):
    x = np.asarray(inputs["x"], dtype=np.float32)
    fingerprints = np.asarray(inputs["fingerprints"], dtype=np.float32)
    cached_outputs = np.asarray(inputs["cached_outputs"], dtype=np.float32)

    nc = build(chunks)
    meta = make_meta(x.reshape(-1)[:4], fingerprints)
    in_maps = []
    for c in range(N_CORES):
        shard = np.ascontiguousarray(cached_outputs[:, c * RS : (c + 1) * RS, :])
        in_maps.append({"meta": meta, "cached": shard})

    res = run_bass_kernel_spmd(
        nc, in_maps, list(range(N_CORES)), trace=trace, **spmd_kwargs
    )
    out = np.concatenate([res.results[c]["out"] for c in range(N_CORES)], axis=0)
    return out.astype(np.float32), res


def kernel(**inputs) -> np.ndarray:
    out, _ = run(inputs, trace=False)
    return out


# revision 7
# speedup vs baseline: 1.2863x; 1.0001x over previous
"""Content-addressed cache-select kernel for Trainium2 (8 NeuronCores, SPMD).

Problem: out = cached_outputs[idx] where idx is the first row of
`fingerprints` (6x4) exactly equal to the first 4 floats of `x`, else 0.

Strategy (row-parallel over 8 cores):
  - Each core receives its 2048-row shard of all 6 cached slabs plus a
    small staged "meta" vector (fingerprints, the replicated probe tiled
    x6, and index weights) packed on the host.
  - On device: one DMA stages meta into SBUF; the vector engine computes
    the first-match index in 6 small int32 ops (bitwise equality ==
    float equality for these inputs; first match wins, no match -> 0);
    the SP engine loads the index into a register and issues
    dynamic-offset DRAM->DRAM DMAs copying the selected 32MB slab shard
    to the output.
  - The copy is SDMA-engine-bound (~3.16us per 64KB descriptor pair per
    engine). Descriptors round-robin across the 16 SDMA engines per DMA
    instruction; the chunk schedule below skews work away from SDMA
    engine 15, which measures ~12% slower (known TRN2 trait).
"""
import numpy as np

import concourse.bass as bass
import concourse.mybir as mybir
from concourse.bass_utils import run_bass_kernel_spmd

N_CASES = 6
ROWS, COLS = 16384, 4096
N_CORES = 8
RS = ROWS // N_CORES  # rows per core

# Copy split between the two HWDGE queues (SP and Activation issue one DMA
# each). Each half lowers to 256 64KB descriptors sprayed uniformly over the
# 16 SDMA engines.
SPLIT_ROWS = RS // 2


def build(split_rows=None):
    split_rows = split_rows or SPLIT_ROWS
    nc = bass.Bass()
    f32 = mybir.dt.float32
    i32 = mybir.dt.int32

    meta = nc.dram_tensor("meta", [1, 64], i32, kind="ExternalInput")
    cached = nc.dram_tensor("cached", [N_CASES, RS, COLS], f32, kind="ExternalInput")
    out = nc.dram_tensor("out", [RS, COLS], f32, kind="ExternalOutput")

    with (
        nc.sbuf_tensor("stage", [1, 128], i32) as stage,
        nc.Block(no_gpsimd_drain=True) as block,
        nc.semaphore("ssem") as ssem,
        nc.semaphore("vsem") as vsem,
        nc.semaphore("bsem") as bsem,
        nc.semaphore("asem") as asem,
    ):

        @block.sync
        def _(sync):
            sync.dma_start(stage[0:1, 0:64], meta[0:1, 0:64]).then_inc(ssem, 16)

        @block.vector
        def _(vector):
            vector.wait_ge(ssem, 16)
            st = stage
            step = [0]

            def chain(inst):
                step[0] += 1
                inst.then_inc(vsem, 1)
                vector.wait_ge(vsem, step[0])

            # eq[64:88] = (fps == probe_tiled) as int32 0/1 (bitwise equality)
            chain(
                vector.tensor_tensor(
                    st[0:1, 64:88],
                    st[0:1, 0:24],
                    st[0:1, 24:48],
                    mybir.AluOpType.is_equal,
                )
            )
            # all4[88:94] = min over each fingerprint's 4 equality bits
            eq_v = st[0:1, 64:88].rearrange("p (a b) -> p a b", a=6)
            chain(
                vector.tensor_reduce(
                    st[0:1, 88:94], eq_v, mybir.AxisListType.X, mybir.AluOpType.min
                )
            )
            # score[94:100] = all4 * [8,7,6,5,4,3] (weights staged at [48:54])
            chain(
                vector.tensor_tensor(
                    st[0:1, 94:100],
                    st[0:1, 88:94],
                    st[0:1, 48:54],
                    mybir.AluOpType.mult,
                )
            )
            # m[100:101] = max(score) = 8 - first_match (0 if no match)
            chain(
                vector.tensor_reduce(
                    st[0:1, 100:101],
                    st[0:1, 94:100],
                    mybir.AxisListType.X,
                    mybir.AluOpType.max,
                )
            )
            # t[101:102] = 8 - m ; idx[102:103] = t & 7 (maps no-match 8 -> 0)
            chain(
                vector.tensor_scalar(
                    st[0:1, 101:102],
                    st[0:1, 100:101],
                    -1,
                    8,
                    mybir.AluOpType.mult,
                    mybir.AluOpType.add,
                )
            )
            chain(
                vector.tensor_scalar(
                    st[0:1, 102:103],
                    st[0:1, 101:102],
                    7,
                    None,
                    mybir.AluOpType.bitwise_and,
                )
            )

        @block.sync
        def _(sync):
            sync.wait_ge(vsem, 6)
            with sync.register("idxr") as idxr:
                sync.reg_load(idxr, stage[0:1, 102:103])
                idx = sync.snap(idxr, min_val=0, max_val=N_CASES - 1)
            rows = slice(0, split_rows)
            sync.dma_start(out[rows, :], cached[bass.ds(idx, 1), rows, :]).then_inc(
                bsem, 16
            )
            sync.wait_ge(bsem, 16)

        @block.scalar
        def _(scalar):
            scalar.wait_ge(vsem, 6)
            with scalar.register("idxa") as idxa:
                scalar.reg_load(idxa, stage[0:1, 102:103])
                idx2 = scalar.snap(idxa, min_val=0, max_val=N_CASES - 1)
            rows = slice(split_rows, RS)
            scalar.dma_start(out[rows, :], cached[bass.ds(idx2, 1), rows, :]).then_inc(
                asem, 16
            )
            scalar.wait_ge(asem, 16)

    return nc


def make_meta(probe, fps):
    buf = np.zeros((1, 64), dtype=np.int32)
    buf[0, 0:24] = fps.reshape(-1).view(np.int32)
    buf[0, 24:48] = np.tile(probe.reshape(-1), 6).view(np.int32)
    buf[0, 48:54] = np.array([8, 7, 6, 5, 4, 3], dtype=np.int32)
    return buf


def run(inputs, trace=False, **spmd_kwargs):
    x = np.asarray(inputs["x"], dtype=np.float32)
    fingerprints = np.asarray(inputs["fingerprints"], dtype=np.float32)
    cached_outputs = np.asarray(inputs["cached_outputs"], dtype=np.float32)

    nc = build()
    meta = make_meta(x.reshape(-1)[:4], fingerprints)
    in_maps = []
    for c in range(N_CORES):
        shard = np.ascontiguousarray(cached_outputs[:, c * RS : (c + 1) * RS, :])
        in_maps.append({"meta": meta, "cached": shard})

    res = run_bass_kernel_spmd(
        nc, in_maps, list(range(N_CORES)), trace=trace, **spmd_kwargs
    )
    out = np.concatenate([res.results[c]["out"] for c in range(N_CORES)], axis=0)
    return out.astype(np.float32), res


def kernel(**inputs) -> np.ndarray:
    out, _ = run(inputs, trace=False)
    return out
